# revision 1
# baseline (speedup 1.0000x reference)
"""Isomorphic feature extraction kernel for Trainium2 (8 NeuronCores).

Math (per batch b, channel c):
  sub[n]    = 5x5 sliding windows of x[b]              (n = 3600 windows)
  A[c,p]    = P_p @ K_c @ P_p^T                        (p = 120 perms)
  dist      = ||sub||^2 - 2<sub,A> + ||A||^2
  feat[n,c] = min_p dist[n,c,p]
  out       = softmax_n(-feat)  flattened to (B, n*c)

Device mapping: one core per (batch, channel-half) pair -> 8 cores.

The 2<sub,A> + ||A||^2 part is a single fp32r matmul with contraction
K=102. fp32r streams 1 row/cycle (vs 4 for fp32) but only keeps ~16
mantissa bits, so both sides are split into exact bf16 hi/residual
pairs (hi = bf16(v), res = v - hi; both fp32r-exact) and the product is
expanded over four K-row groups: xh*ah + xr*ah + xh*ar + xr*ar.
  lhsT rows   0..24 xh windows   25..49 xr windows
            50..74 xh windows   75..99 xr windows   100,101 ones
  rhs  rows   0..24 hi(2A)       25..49 hi(2A)
            50..74 res(2A)      75..99 res(2A)
            100 -hi(||A||^2)    101 -res(||A||^2)
psum[n,(c,p)] = 2<sub,A> - ||A||^2.  ||sub||^2 is exact fp32 via a tiny
N=1 matmul (x^2 windows x -ones) into a spare psum pad column, copied
(ScalarE) to SBUF and applied as the bias of exp on ScalarE.

feat comes from one fused reduce_max over (bank, p) per 128-row tile;
softmax denominator = ones-vector matmul (partition-dim sum), then
reciprocal + broadcast multiply. Pad rows (n in [3600,3712)) see
-25e6 in the nx bias, so exp underflows to 0 and they drop out of the
denominator.
"""

import numpy as np
import ml_dtypes
from itertools import permutations

import concourse.bacc as bacc
import concourse.mybir as mybir
from concourse import tile
from concourse import bass_utils

F32 = mybir.dt.float32
F32R = mybir.dt.float32r

B = 4
KS = 5
NH = 60
N = NH * NH            # 3600 subgraph windows
NT = 29                # 128-row tiles over n
NPAD = NT * 128        # 3712
PADW = NPAD - N        # 112
C = 32
CH = 16                # channels per core
NPERM = 120
NBANK = 4              # psum banks per tile
PL = NPERM // NBANK    # 30 perms per bank
KDIM = 102             # 4 x 25 split-product rows + 2 ones rows
BANKF = 512            # fp32 elems per psum bank
COLS = CH * PL         # 480 used columns per bank
NXCOL = COLS           # pad column in bank 0 holding -||sub||^2

_CACHE = {}


def _bf16(a):
    return a.astype(ml_dtypes.bfloat16).astype(np.float32)


def _perm_mats():
    perms = list(permutations(range(KS)))
    P = np.zeros((len(perms), KS, KS), dtype=np.float32)
    for idx, p in enumerate(perms):
        P[idx, np.arange(KS), np.array(p)] = 1.0
    return P


def _col_order(a):
    """(16, 120, 25) -> (25, 1920) in column order bank*480 + c*30 + pl."""
    return a.reshape(CH, NBANK, PL, 25).transpose(3, 1, 0, 2).reshape(25, NBANK * COLS)


def _make_rhs(kernel1, Pm, h):
    A = np.einsum("pik,ckl,pjl->cpij", Pm, kernel1, Pm)  # (32, 120, 5, 5) fp32
    A = A[h * CH:(h + 1) * CH]                           # (16, 120, 5, 5)
    nA = (A.astype(np.float64) ** 2).sum(axis=(-2, -1)).astype(np.float32)  # (16,120)
    A2 = 2.0 * A.reshape(CH, NPERM, 25)
    ah = _bf16(A2)
    ar = A2 - ah
    nAh = _bf16(nA)
    nAr = nA - nAh
    rhs = np.empty((KDIM, NBANK * COLS), dtype=np.float32)
    rhs[0:25] = _col_order(ah)
    rhs[25:50] = rhs[0:25]
    rhs[50:75] = _col_order(ar)
    rhs[75:100] = rhs[50:75]
    rhs[100] = -nAh.reshape(CH, NBANK, PL).transpose(1, 0, 2).reshape(-1)
    rhs[101] = -nAr.reshape(CH, NBANK, PL).transpose(1, 0, 2).reshape(-1)
    return rhs


def _body(nc, tc, lhs, l2, rhs, out):
    ACT = mybir.ActivationFunctionType
    with tc.tile_pool(name="const", bufs=1) as cp:
        L = cp.tile([128, NPAD], F32R)
        L2 = cp.tile([25, NPAD], F32)
        R = cp.tile([KDIM, NBANK * COLS], F32R)
        m1 = cp.tile([25, 1], F32)
        feat = cp.tile([128, NT * CH], F32)
        negnx = cp.tile([128, NT], F32)
        E = cp.tile([128, NT * CH], F32)
        onescol = cp.tile([128, 1], F32)
        ones1 = cp.tile([1, 128], F32)
        densum = cp.tile([1, CH], F32)
        recip = cp.tile([1, CH], F32)
        outsb = cp.tile([128, NT * CH], F32)
        warm = cp.tile([1, 4], F32)

        # preload the exp LUT set while the main loop runs
        nc.vector.memset(warm[:, :], 0.0)
        nc.scalar.activation(warm[:, :], warm[:, :], ACT.Exp)

        nc.vector.memset(onescol[:, :], 1.0)
        nc.vector.memset(ones1[:, :], 1.0)
        nc.vector.memset(m1[:, :], -1.0)

        # lhsT matrices are repacked on host (im2col of the 16KB input) so
        # they land as large contiguous DMAs instead of 125 tiny strided
        # ones (240B runs cost ~10us each in DMA descriptors). Chunked so
        # early tiles' matmuls start before the whole upload finishes and
        # the chunks spread across HW DMA queues.
        for b in range(NBANK):
            nc.sync.dma_start(
                out=R[:, b * COLS:(b + 1) * COLS],
                in_=rhs[:, b * COLS:(b + 1) * COLS],
            )
        CHUNK = NPAD // 4
        for k in range(4):
            lo, hi = k * CHUNK, (k + 1) * CHUNK
            nc.sync.dma_start(out=L[0:KDIM, lo:hi], in_=lhs[:, lo:hi])
            nc.sync.dma_start(out=L2[0:25, lo:hi], in_=l2[:, lo:hi])

        with tc.tile_pool(name="psum", bufs=2, space="PSUM") as pp:
            for t in range(NT):
                ps = pp.tile([128, NBANK * BANKF], F32, tag="ps")
                lt = L[0:KDIM, t * 128:(t + 1) * 128]
                for b in range(NBANK):
                    nc.tensor.matmul(
                        ps[:, b * BANKF:b * BANKF + COLS],
                        lt,
                        R[:, b * COLS:(b + 1) * COLS],
                    )
                # -||sub||^2 into the pad column of bank 0 (exact fp32 path)
                nc.tensor.matmul(
                    ps[:, NXCOL:NXCOL + 1],
                    L2[0:25, t * 128:(t + 1) * 128],
                    m1[:, :],
                )
                nc.scalar.copy(negnx[:, t:t + 1], ps[:, NXCOL:NXCOL + 1])
                v = (
                    ps[:, :]
                    .rearrange("q (b s) -> q b s", s=BANKF)[:, :, 0:COLS]
                    .rearrange("q b (c p) -> q b c p", p=PL)
                    .transpose([0, 2, 1, 3])
                )
                nc.vector.reduce_max(
                    feat[:, t * CH:(t + 1) * CH], v, axis=mybir.AxisListType.XY
                )

        for t in range(NT):
            nc.scalar.activation(
                E[:, t * CH:(t + 1) * CH],
                feat[:, t * CH:(t + 1) * CH],
                ACT.Exp,
                bias=negnx[:, t:t + 1],
            )

        with tc.tile_pool(name="psum2", bufs=1, space="PSUM") as pp2:
            den = pp2.tile([1, BANKF], F32, tag="den")
            nc.tensor.matmul(den[0:1, 0:NT * CH], onescol[:, :], E[:, :])
            dv = den[0:1, 0:NT * CH].rearrange("q (t c) -> q t c", c=CH).transpose([0, 2, 1])
            nc.vector.reduce_sum(densum[:, :], dv, axis=mybir.AxisListType.X)
            nc.vector.reciprocal(recip[:, :], densum[:, :])
            bc = pp2.tile([128, CH], F32, tag="bc")
            nc.tensor.matmul(bc[:, :], ones1[:, :], recip[:, :])
            Ev = E[:, :].rearrange("q (t c) -> q t c", c=CH)
            bv = bc[:, :].unsqueeze(1).broadcast_to((128, NT, CH))
            ov = outsb[:, :].rearrange("q (t c) -> q t c", c=CH)
            nc.vector.tensor_mul(out=ov, in0=Ev, in1=bv)

        nc.sync.dma_start(out=out.transpose([1, 0, 2]), in_=outsb[:, :])


def _build():
    if "nc" in _CACHE:
        return _CACHE["nc"]
    nc = bacc.Bacc("TRN2", target_bir_lowering=False, debug=False, num_devices=8)
    lhs_d = nc.declare_dram_parameter("lhs", [KDIM, NPAD], F32R, isOutput=False)
    l2_d = nc.declare_dram_parameter("l2", [25, NPAD], F32, isOutput=False)
    rhs_d = nc.declare_dram_parameter("rhs", [KDIM, NBANK * COLS], F32R, isOutput=False)
    out_d = nc.declare_dram_parameter("out", [NT, 128, CH], F32, isOutput=True)
    with tile.TileContext(nc) as tc:
        _body(nc, tc, lhs_d.ap(), l2_d.ap(), rhs_d.ap(), out_d.ap())
    nc.compile()
    _CACHE["nc"] = nc
    return nc


def _im2col(img):
    """(64,64) -> (25, 3712) window rows; pad columns zero."""
    w = np.lib.stride_tricks.sliding_window_view(img, (KS, KS))  # (60,60,5,5)
    out = np.zeros((25, NPAD), dtype=np.float32)
    out[:, :N] = w.reshape(N, 25).T
    return out


def make_in_maps(x, kernel1, P):
    x = np.asarray(x, dtype=np.float32)
    kernel1 = np.asarray(kernel1, dtype=np.float32)
    rhs_halves = [_make_rhs(kernel1, _perm_mats(), h) for h in range(2)]
    in_maps = []
    for core in range(8):
        b, h = core // 2, core % 2
        xb = np.ascontiguousarray(x[b])
        xbh = _bf16(xb)
        lhs = np.zeros((KDIM, NPAD), dtype=np.float32)
        lhs[0:25] = _im2col(xbh)
        lhs[25:50] = _im2col(xb - xbh)
        lhs[50:75] = lhs[0:25]
        lhs[75:100] = lhs[25:50]
        lhs[100:102, :N] = 1.0            # ones rows; pad cols stay 0
        l2 = np.full((25, NPAD), 1.0e6, dtype=np.float32)
        l2[:, :N] = _im2col(xb * xb)[:, :N]
        in_maps.append({"lhs": lhs, "l2": l2, "rhs": rhs_halves[h]})
    return in_maps


def assemble(results):
    full = np.empty((B, N, C), dtype=np.float32)
    for core in range(8):
        b, h = core // 2, core % 2
        o = np.asarray(results[core]["out"]).reshape(NPAD, CH)[:N]
        full[b, :, h * CH:(h + 1) * CH] = o
    return full.reshape(B, -1)


def kernel(x, kernel1, P):
    nc = _build()
    in_maps = make_in_maps(x, kernel1, P)
    res = bass_utils.run_bass_kernel_spmd(nc, in_maps, core_ids=list(range(8)))
    return assemble(res.results)



# revision 15
# speedup vs baseline: 1.8252x; 1.8252x over previous
"""Isomorphic feature extraction kernel for Trainium2 (8 NeuronCores).

Math (per batch b, channel c):
  sub[n]    = 5x5 sliding windows of x[b]              (n = 3600 windows)
  A[c,p]    = P_p @ K_c @ P_p^T                        (p = 120 perms)
  dist      = ||sub||^2 - 2<sub,A> + ||A||^2           (||A||^2 = ||K_c||^2, perm-invariant)
  feat[n,c] = min_p dist[n,c,p]
  out       = softmax_n(-feat)  flattened to (B, n*c)

Device mapping: one core per (batch, channel-half) pair -> 8 cores.

Let v_p = 2<sub,A_p> - ||K_c||^2 + C - ||sub||^2 (C = 1000), so
feat = C - max_p v_p and the softmax numerator is E = exp(max_p v - C).
Each v_p is one fp32r matmul column with a 127-row contraction: 4 x 25
exact bf16 hi/residual split-product rows for 2<sub,A>, 2 rows for
(C - ||K_c||^2), and 25 im2col(x^2) rows against -1.

The 120-way max is halved on the PE before DVE ever sees it, via
  max(v_i, v_j) = (v_i + v_j)/2 + |v_i - v_j|/2.
Both (v_i+v_j)/2 ("s") and (v_i-v_j)/2 ("d") are single matmul columns
with host-precomputed rhs (60 pairs per channel each). Per window-tile:
PSUM tile X = s-cols (2 banks: channels 0-7 | 8-15), tile Y = d-cols.
ScalarE computes r = |Y| into SBUF; the PE accumulates r onto X with an
identity-weight fp32r matmul (start=False), software-pipelined one tile
behind so the PE never waits on ScalarE. DVE then reduce_maxes only 60
values per channel. One 32-wide exp per tile-pair; E is DMAed out in 3
chunks as exps complete; softmax normalization (a per-(batch,channel)
scalar divide) happens on host during unshard. Pad windows have
all-zero lhs columns -> v = 0 -> exp(-C) = 0.
"""

import numpy as np
import ml_dtypes
from itertools import permutations

import concourse.bacc as bacc
import concourse.mybir as mybir
from concourse import tile
from concourse import bass_utils

F32 = mybir.dt.float32
F32R = mybir.dt.float32r

B = 4
KS = 5
NH = 60
N = NH * NH            # 3600 subgraph windows
NT = 29                # 128-row tiles over n
NPAD = NT * 128        # 3712
C = 32
CH = 16                # channels per core
HCH = 8                # channels per psum bank
NPERM = 120
NPAIR = 60             # perm pairs per channel
KDIM = 127             # 4x25 split rows + 2 shift/norm rows + 25 x^2 rows
BANKF = 512            # fp32 elems per psum bank
HWID = 2 * BANKF       # 1024 cols per 2-bank psum tile
SBLK = HCH * NPAIR     # 480 used cols per bank
RWID = 4 * SBLK        # 1920 rhs cols: [s-h0][s-h1][d-h0][d-h1]
SHIFT = 1000.0         # additive shift keeping psum values positive

_CACHE = {}


def _bf16(a):
    return a.astype(ml_dtypes.bfloat16).astype(np.float32)


def _perm_mats():
    perms = list(permutations(range(KS)))
    P = np.zeros((len(perms), KS, KS), dtype=np.float32)
    for idx, p in enumerate(perms):
        P[idx, np.arange(KS), np.array(p)] = 1.0
    return P


def _fill_block(rhs, col0, G, u, x2coef):
    """Fill one 480-col rhs block: G (8, 60, 25) column vectors, u (8,)
    norm/shift row value, x2coef the x^2-row coefficient."""
    g = G.reshape(SBLK, 25).T.astype(np.float32)        # (25, 480)
    gh = _bf16(g)
    rhs[0:25, col0:col0 + SBLK] = gh
    rhs[25:50, col0:col0 + SBLK] = gh
    rhs[50:75, col0:col0 + SBLK] = g - gh
    rhs[75:100, col0:col0 + SBLK] = rhs[50:75, col0:col0 + SBLK]
    uc = np.repeat(u.astype(np.float32), NPAIR)         # (480,)
    uh = _bf16(uc)
    rhs[100, col0:col0 + SBLK] = uh
    rhs[101, col0:col0 + SBLK] = uc - uh
    rhs[102:127, col0:col0 + SBLK] = x2coef


def _make_rhs(kernel1, Pm, h):
    A = np.einsum("pik,ckl,pjl->cpij", Pm, kernel1, Pm)  # (32, 120, 5, 5)
    A = A[h * CH:(h + 1) * CH].astype(np.float64)        # (16, 120, 5, 5)
    nA = (A[:, 0] ** 2).sum(axis=(-2, -1))               # (16,) perm-invariant
    A2 = 2.0 * A.reshape(CH, NPERM, 25)
    s = (A2[:, 0::2] + A2[:, 1::2]) / 2.0                # (16, 60, 25)
    d = (A2[:, 0::2] - A2[:, 1::2]) / 2.0
    u = (SHIFT - nA)                                     # (16,)
    z = np.zeros(HCH)
    rhs = np.zeros((KDIM, RWID), dtype=np.float32)
    _fill_block(rhs, 0 * SBLK, s[0:HCH], u[0:HCH], -1.0)
    _fill_block(rhs, 1 * SBLK, s[HCH:CH], u[HCH:CH], -1.0)
    _fill_block(rhs, 2 * SBLK, d[0:HCH], z, 0.0)
    _fill_block(rhs, 3 * SBLK, d[HCH:CH], z, 0.0)
    return rhs


def _body(nc, tc, lhs, rhs, eye, out):
    ACT = mybir.ActivationFunctionType
    with tc.tile_pool(name="const", bufs=1) as cp:
        L = cp.tile([128, NPAD], F32R)
        R = cp.tile([KDIM, RWID], F32R)
        I = cp.tile([128, 128], F32R)
        negC = cp.tile([128, 1], F32)
        feat = cp.tile([128, NT * CH], F32)
        E = cp.tile([128, NT * CH], F32)
        warm = cp.tile([1, 4], F32)

        nc.vector.memset(negC[:, :], -SHIFT)

        # Input DMAs spread over the Pool/Activation/SP queues, tile-0's
        # operands (L cols 0-463, s/d blocks for half 0) first on each.
        C0 = 464
        nc.sync.dma_start(out=L[0:KDIM, 0:C0], in_=lhs[:, 0:C0])
        nc.gpsimd.dma_start(out=R[:, 0:SBLK], in_=rhs[:, 0:SBLK])
        nc.scalar.dma_start(out=R[:, 2 * SBLK:3 * SBLK], in_=rhs[:, 2 * SBLK:3 * SBLK])
        nc.gpsimd.dma_start(out=R[:, SBLK:2 * SBLK], in_=rhs[:, SBLK:2 * SBLK])
        nc.scalar.dma_start(out=R[:, 3 * SBLK:RWID], in_=rhs[:, 3 * SBLK:RWID])
        nc.gpsimd.dma_start(out=I[:, :], in_=eye)
        CH1 = (NPAD - C0) // 3
        for k in range(3):
            lo = C0 + k * CH1
            hi = NPAD if k == 2 else lo + CH1
            nc.sync.dma_start(out=L[0:KDIM, lo:hi], in_=lhs[:, lo:hi])

        # preload the exp/abs LUT set while the DMAs run
        nc.vector.memset(warm[:, :], 0.0)
        nc.scalar.activation(warm[:, :], warm[:, :], ACT.Exp)

        def emit_finish(t, X):
            """I-add of |d| onto the s-banks, reduce, exp (per tile-pair),
            output DMA (per chunk)."""
            r = rtiles[t]
            nc.tensor.matmul(X[:, 0:SBLK], I[:, :], r[:, 0:SBLK],
                             start=False, stop=True)
            nc.tensor.matmul(X[:, BANKF:BANKF + SBLK], I[:, :],
                             r[:, SBLK:2 * SBLK], start=False, stop=True)
            v = (X[:, :].rearrange("q (b s) -> q b s", s=BANKF)
                 [:, :, 0:SBLK]
                 .rearrange("q b (c p) -> q b c p", p=NPAIR))
            fv = (feat[:, t * CH:(t + 1) * CH]
                  .rearrange("q (b c) -> q b c", c=HCH))
            nc.vector.reduce_max(fv, v, axis=mybir.AxisListType.X)
            if t % 2 == 1 or t == NT - 1:
                lo = t - 1 if t % 2 == 1 else t
                nc.scalar.activation(
                    E[:, lo * CH:(t + 1) * CH],
                    feat[:, lo * CH:(t + 1) * CH],
                    ACT.Exp, bias=negC[:, :],
                )
            if t + 1 in (12, 22, 28, NT):
                lo = {12: 0, 22: 12, 28: 22, NT: 28}[t + 1]
                nc.sync.dma_start(out=out[:, lo:t + 1, :],
                                  in_=E[:, lo * CH:(t + 1) * CH])

        rtiles = {}
        with tc.tile_pool(name="psum", bufs=2, space="PSUM") as pp:
            prev = None
            for t in range(NT):
                lt = L[0:KDIM, t * 128:(t + 1) * 128]
                X = pp.tile([128, HWID], F32, tag="px", bufs=2)
                Y = pp.tile([128, HWID], F32, tag="py", bufs=2)
                nc.tensor.matmul(X[:, 0:SBLK], lt, R[:, 0:SBLK],
                                 start=True, stop=False)
                nc.tensor.matmul(X[:, BANKF:BANKF + SBLK], lt,
                                 R[:, SBLK:2 * SBLK], start=True, stop=False)
                nc.tensor.matmul(Y[:, 0:SBLK], lt, R[:, 2 * SBLK:3 * SBLK])
                nc.tensor.matmul(Y[:, BANKF:BANKF + SBLK], lt,
                                 R[:, 3 * SBLK:RWID])
                # r = |d| on ScalarE; consumed by next tile's I-add
                r = cp.tile([128, 2 * SBLK], F32R, tag="r", bufs=3)
                rtiles[t] = r
                yv = (Y[:, :].rearrange("q (b s) -> q b s", s=BANKF)
                      [:, :, 0:SBLK])
                rv = r[:, :].rearrange("q (b s) -> q b s", s=SBLK)
                nc.scalar.activation(rv, yv, ACT.Abs)
                if prev is not None:
                    emit_finish(t - 1, prev)
                prev = X
            emit_finish(NT - 1, prev)


def _build():
    if "nc" in _CACHE:
        return _CACHE["nc"]
    nc = bacc.Bacc("TRN2", target_bir_lowering=False, debug=False, num_devices=8)
    lhs_d = nc.declare_dram_parameter("lhs", [KDIM, NPAD], F32R, isOutput=False)
    rhs_d = nc.declare_dram_parameter("rhs", [KDIM, RWID], F32R, isOutput=False)
    eye_d = nc.declare_dram_parameter("eye", [128, 128], F32R, isOutput=False)
    out_d = nc.declare_dram_parameter("out", [128, NT, CH], F32, isOutput=True)
    with tile.TileContext(nc) as tc:
        _body(nc, tc, lhs_d.ap(), rhs_d.ap(), eye_d.ap(), out_d.ap())
    nc.compile()
    _CACHE["nc"] = nc
    return nc


def _im2col(img):
    """(64,64) -> (25, 3712) window rows; pad columns zero."""
    w = np.lib.stride_tricks.sliding_window_view(img, (KS, KS))  # (60,60,5,5)
    out = np.zeros((25, NPAD), dtype=np.float32)
    out[:, :N] = w.reshape(N, 25).T
    return out


def make_in_maps(x, kernel1, P):
    x = np.asarray(x, dtype=np.float32)
    kernel1 = np.asarray(kernel1, dtype=np.float32)
    rhs_halves = [_make_rhs(kernel1, _perm_mats(), h) for h in range(2)]
    eye = np.eye(128, dtype=np.float32)
    in_maps = []
    for core in range(8):
        b, h = core // 2, core % 2
        xb = np.ascontiguousarray(x[b])
        xbh = _bf16(xb)
        lhs = np.zeros((KDIM, NPAD), dtype=np.float32)
        lhs[0:25] = _im2col(xbh)
        lhs[25:50] = _im2col(xb - xbh)
        lhs[50:75] = lhs[0:25]
        lhs[75:100] = lhs[25:50]
        lhs[100:102, :N] = 1.0            # shift/norm rows; pad cols stay 0
        lhs[102:127] = _im2col(xb * xb)   # x^2 rows (pad cols zero)
        in_maps.append({"lhs": lhs, "rhs": rhs_halves[h], "eye": eye})
    return in_maps


def assemble(results):
    full = np.empty((B, N, C), dtype=np.float32)
    for core in range(8):
        b, h = core // 2, core % 2
        o = np.asarray(results[core]["out"])          # (128, NT, 16) = E
        e = o.transpose(1, 0, 2).reshape(NPAD, CH)[:N].astype(np.float64)
        full[b, :, h * CH:(h + 1) * CH] = (e / e.sum(axis=0)).astype(np.float32)
    return full.reshape(B, -1)


def kernel(x, kernel1, P):
    nc = _build()
    in_maps = make_in_maps(x, kernel1, P)
    res = bass_utils.run_bass_kernel_spmd(nc, in_maps, core_ids=list(range(8)))
    return assemble(res.results)


# revision 24
# speedup vs baseline: 1.8259x; 1.0004x over previous
"""Isomorphic feature extraction kernel for Trainium2 (8 NeuronCores).

Math (per batch b, channel c):
  sub[n]    = 5x5 sliding windows of x[b]              (n = 3600 windows)
  A[c,p]    = P_p @ K_c @ P_p^T                        (p = 120 perms)
  dist      = ||sub||^2 - 2<sub,A> + ||A||^2           (||A||^2 = ||K_c||^2, perm-invariant)
  feat[n,c] = min_p dist[n,c,p]
  out       = softmax_n(-feat)  flattened to (B, n*c)

Device mapping: one core per (batch, channel-half) pair -> 8 cores.

Let v_p = 2<sub,A_p> - ||K_c||^2 + C - ||sub||^2 (C = 1000), so
feat = C - max_p v_p and the softmax numerator is E = exp(max_p v - C).
Each v_p is one fp32r matmul column with a 127-row contraction: 4 x 25
exact bf16 hi/residual split-product rows for 2<sub,A>, 2 rows for
(C - ||K_c||^2), and 25 im2col(x^2) rows against -1.

The 120-way max is halved on the PE before DVE ever sees it, via
  max(v_i, v_j) = (v_i + v_j)/2 + |v_i - v_j|/2.
Both (v_i+v_j)/2 ("s") and (v_i-v_j)/2 ("d") are single matmul columns
with host-precomputed rhs (60 pairs per channel each). Per window-tile:
PSUM tile X = s-cols (2 banks: channels 0-7 | 8-15), tile Y = d-cols.
ScalarE computes r = |Y| into SBUF; the PE accumulates r onto X with an
identity-weight fp32r matmul (start=False), software-pipelined one tile
behind so the PE never waits on ScalarE. DVE then reduce_maxes only 60
values per channel. One 32-wide exp per tile-pair; E is DMAed out in 3
chunks as exps complete; softmax normalization (a per-(batch,channel)
scalar divide) happens on host during unshard. Pad windows have
all-zero lhs columns -> v = 0 -> exp(-C) = 0.
"""

import numpy as np
import ml_dtypes
from itertools import permutations

import concourse.bacc as bacc
import concourse.mybir as mybir
from concourse import tile
from concourse import bass_utils

F32 = mybir.dt.float32
F32R = mybir.dt.float32r

B = 4
KS = 5
NH = 60
N = NH * NH            # 3600 subgraph windows
NT = 29                # 128-row tiles over n
NPAD = NT * 128        # 3712
C = 32
CH = 16                # channels per core
HCH = 8                # channels per psum bank
NPERM = 120
NPAIR = 60             # perm pairs per channel
KDIM = 127             # 4x25 split rows + 2 shift/norm rows + 25 x^2 rows
BANKF = 512            # fp32 elems per psum bank
HWID = 2 * BANKF       # 1024 cols per 2-bank psum tile
SBLK = HCH * NPAIR     # 480 used cols per bank
RWID = 4 * SBLK        # 1920 rhs cols: [s-h0][s-h1][d-h0][d-h1]
SHIFT = 1000.0         # additive shift keeping psum values positive

_CACHE = {}


def _bf16(a):
    return a.astype(ml_dtypes.bfloat16).astype(np.float32)


def _perm_mats():
    perms = list(permutations(range(KS)))
    P = np.zeros((len(perms), KS, KS), dtype=np.float32)
    for idx, p in enumerate(perms):
        P[idx, np.arange(KS), np.array(p)] = 1.0
    return P


def _fill_block(rhs, col0, G, u, x2coef):
    """Fill one 480-col rhs block: G (8, 60, 25) column vectors, u (8,)
    norm/shift row value, x2coef the x^2-row coefficient."""
    g = G.reshape(SBLK, 25).T.astype(np.float32)        # (25, 480)
    gh = _bf16(g)
    rhs[0:25, col0:col0 + SBLK] = gh
    rhs[25:50, col0:col0 + SBLK] = gh
    rhs[50:75, col0:col0 + SBLK] = g - gh
    rhs[75:100, col0:col0 + SBLK] = rhs[50:75, col0:col0 + SBLK]
    uc = np.repeat(u.astype(np.float32), NPAIR)         # (480,)
    uh = _bf16(uc)
    rhs[100, col0:col0 + SBLK] = uh
    rhs[101, col0:col0 + SBLK] = uc - uh
    rhs[102:127, col0:col0 + SBLK] = x2coef


def _make_rhs(kernel1, Pm, h):
    A = np.einsum("pik,ckl,pjl->cpij", Pm, kernel1, Pm)  # (32, 120, 5, 5)
    A = A[h * CH:(h + 1) * CH].astype(np.float64)        # (16, 120, 5, 5)
    nA = (A[:, 0] ** 2).sum(axis=(-2, -1))               # (16,) perm-invariant
    A2 = 2.0 * A.reshape(CH, NPERM, 25)
    s = (A2[:, 0::2] + A2[:, 1::2]) / 2.0                # (16, 60, 25)
    d = (A2[:, 0::2] - A2[:, 1::2]) / 2.0
    u = (SHIFT - nA)                                     # (16,)
    z = np.zeros(HCH)
    rhs = np.zeros((KDIM, RWID), dtype=np.float32)
    _fill_block(rhs, 0 * SBLK, s[0:HCH], u[0:HCH], -1.0)
    _fill_block(rhs, 1 * SBLK, s[HCH:CH], u[HCH:CH], -1.0)
    _fill_block(rhs, 2 * SBLK, d[0:HCH], z, 0.0)
    _fill_block(rhs, 3 * SBLK, d[HCH:CH], z, 0.0)
    return rhs


def _body(nc, tc, lhs, rhs, eye, out):
    ACT = mybir.ActivationFunctionType
    with tc.tile_pool(name="const", bufs=1) as cp:
        L = cp.tile([128, NPAD], F32R)
        R = cp.tile([KDIM, RWID], F32R)
        I = cp.tile([128, 128], F32R)
        negC = cp.tile([128, 1], F32)
        feat = cp.tile([128, NT * CH], F32)
        E = cp.tile([128, NT * CH], F32)
        warm = cp.tile([1, 4], F32)

        nc.vector.memset(negC[:, :], -SHIFT)

        # Input DMAs spread over the Pool/Activation/SP queues, tile-0's
        # operands (L cols 0-463, s/d blocks for half 0) first on each.
        C0 = 464
        nc.sync.dma_start(out=L[0:KDIM, 0:C0 // 2], in_=lhs[:, 0:C0 // 2])
        nc.sync.dma_start(out=L[0:KDIM, C0 // 2:C0], in_=lhs[:, C0 // 2:C0])
        nc.gpsimd.dma_start(out=R[:, 0:SBLK], in_=rhs[:, 0:SBLK])
        nc.scalar.dma_start(out=R[:, 2 * SBLK:3 * SBLK], in_=rhs[:, 2 * SBLK:3 * SBLK])
        nc.gpsimd.dma_start(out=R[:, SBLK:2 * SBLK], in_=rhs[:, SBLK:2 * SBLK])
        nc.scalar.dma_start(out=R[:, 3 * SBLK:RWID], in_=rhs[:, 3 * SBLK:RWID])
        nc.gpsimd.dma_start(out=I[:, :], in_=eye)
        CH1 = (NPAD - C0) // 3
        for k in range(3):
            lo = C0 + k * CH1
            hi = NPAD if k == 2 else lo + CH1
            nc.sync.dma_start(out=L[0:KDIM, lo:hi], in_=lhs[:, lo:hi])

        # preload the exp/abs LUT set while the DMAs run
        nc.vector.memset(warm[:, :], 0.0)
        nc.scalar.activation(warm[:, :], warm[:, :], ACT.Exp)

        def emit_finish(t, X):
            """I-add of |d| onto the s-banks, reduce, exp (per tile-pair),
            output DMA (per chunk)."""
            r = rtiles[t]
            nc.tensor.matmul(X[:, 0:SBLK], I[:, :], r[:, 0:SBLK],
                             start=False, stop=True)
            nc.tensor.matmul(X[:, BANKF:BANKF + SBLK], I[:, :],
                             r[:, SBLK:2 * SBLK], start=False, stop=True)
            v = (X[:, :].rearrange("q (b s) -> q b s", s=BANKF)
                 [:, :, 0:SBLK]
                 .rearrange("q b (c p) -> q b c p", p=NPAIR))
            fv = (feat[:, t * CH:(t + 1) * CH]
                  .rearrange("q (b c) -> q b c", c=HCH))
            nc.vector.reduce_max(fv, v, axis=mybir.AxisListType.X)
            if t % 2 == 1 or t == NT - 1:
                lo = t - 1 if t % 2 == 1 else t
                nc.scalar.activation(
                    E[:, lo * CH:(t + 1) * CH],
                    feat[:, lo * CH:(t + 1) * CH],
                    ACT.Exp, bias=negC[:, :],
                )
            if t + 1 in (12, 22, 28, NT):
                lo = {12: 0, 22: 12, 28: 22, NT: 28}[t + 1]
                nc.sync.dma_start(out=out[:, lo:t + 1, :],
                                  in_=E[:, lo * CH:(t + 1) * CH])

        rtiles = {}
        with tc.tile_pool(name="psum", bufs=2, space="PSUM") as pp:
            prev = None
            for t in range(NT):
                lt = L[0:KDIM, t * 128:(t + 1) * 128]
                X = pp.tile([128, HWID], F32, tag="px", bufs=2)
                Y = pp.tile([128, HWID], F32, tag="py", bufs=2)
                nc.tensor.matmul(X[:, 0:SBLK], lt, R[:, 0:SBLK],
                                 start=True, stop=False)
                nc.tensor.matmul(X[:, BANKF:BANKF + SBLK], lt,
                                 R[:, SBLK:2 * SBLK], start=True, stop=False)
                nc.tensor.matmul(Y[:, 0:SBLK], lt, R[:, 2 * SBLK:3 * SBLK])
                nc.tensor.matmul(Y[:, BANKF:BANKF + SBLK], lt,
                                 R[:, 3 * SBLK:RWID])
                # r = |d| on ScalarE; consumed by next tile's I-add
                r = cp.tile([128, 2 * SBLK], F32R, tag="r", bufs=3)
                rtiles[t] = r
                yv = (Y[:, :].rearrange("q (b s) -> q b s", s=BANKF)
                      [:, :, 0:SBLK])
                rv = r[:, :].rearrange("q (b s) -> q b s", s=SBLK)
                nc.scalar.activation(rv, yv, ACT.Abs)
                if prev is not None:
                    emit_finish(t - 1, prev)
                prev = X
            emit_finish(NT - 1, prev)


def _build():
    if "nc" in _CACHE:
        return _CACHE["nc"]
    nc = bacc.Bacc("TRN2", target_bir_lowering=False, debug=False, num_devices=8)
    lhs_d = nc.declare_dram_parameter("lhs", [KDIM, NPAD], F32R, isOutput=False)
    rhs_d = nc.declare_dram_parameter("rhs", [KDIM, RWID], F32R, isOutput=False)
    eye_d = nc.declare_dram_parameter("eye", [128, 128], F32R, isOutput=False)
    out_d = nc.declare_dram_parameter("out", [128, NT, CH], F32, isOutput=True)
    with tile.TileContext(nc) as tc:
        _body(nc, tc, lhs_d.ap(), rhs_d.ap(), eye_d.ap(), out_d.ap())
    nc.compile()
    _CACHE["nc"] = nc
    return nc


def _im2col(img):
    """(64,64) -> (25, 3712) window rows; pad columns zero."""
    w = np.lib.stride_tricks.sliding_window_view(img, (KS, KS))  # (60,60,5,5)
    out = np.zeros((25, NPAD), dtype=np.float32)
    out[:, :N] = w.reshape(N, 25).T
    return out


def make_in_maps(x, kernel1, P):
    x = np.asarray(x, dtype=np.float32)
    kernel1 = np.asarray(kernel1, dtype=np.float32)
    rhs_halves = [_make_rhs(kernel1, _perm_mats(), h) for h in range(2)]
    eye = np.eye(128, dtype=np.float32)
    in_maps = []
    for core in range(8):
        b, h = core // 2, core % 2
        xb = np.ascontiguousarray(x[b])
        xbh = _bf16(xb)
        lhs = np.zeros((KDIM, NPAD), dtype=np.float32)
        lhs[0:25] = _im2col(xbh)
        lhs[25:50] = _im2col(xb - xbh)
        lhs[50:75] = lhs[0:25]
        lhs[75:100] = lhs[25:50]
        lhs[100:102, :N] = 1.0            # shift/norm rows; pad cols stay 0
        lhs[102:127] = _im2col(xb * xb)   # x^2 rows (pad cols zero)
        in_maps.append({"lhs": lhs, "rhs": rhs_halves[h], "eye": eye})
    return in_maps


def assemble(results):
    full = np.empty((B, N, C), dtype=np.float32)
    for core in range(8):
        b, h = core // 2, core % 2
        o = np.asarray(results[core]["out"])          # (128, NT, 16) = E
        e = o.transpose(1, 0, 2).reshape(NPAD, CH)[:N].astype(np.float64)
        full[b, :, h * CH:(h + 1) * CH] = (e / e.sum(axis=0)).astype(np.float32)
    return full.reshape(B, -1)


def kernel(x, kernel1, P):
    nc = _build()
    in_maps = make_in_maps(x, kernel1, P)
    res = bass_utils.run_bass_kernel_spmd(nc, in_maps, core_ids=list(range(8)))
    return assemble(res.results)
